# revision 1
# baseline (speedup 1.0000x reference)
"""HTSAD (event-filtered peephole LSTM) Trainium2 kernel.

Strategy: data-parallel over batch (B=64 -> 8 cores x B_LOC=8), sequential
scan over S=4096 on each core.

Per-core layout is fully transposed (feature dims on SBUF partitions, batch
on the free dim):
  - gates PSUM: 8 banks of [128, 8 steps, 8 blocks, 8 batch]; block order
    [f_h0 f_h1 i_h0 i_h1 g_h0 g_h1 o_h0 o_h1] (h = hidden half of HS=256).
  - per micro-chunk of 64 steps: batched matmuls compute x (event/vc/vn
    projections), j-gate, then bias + x@Wx are pre-accumulated into the
    gates PSUM; the scan accumulates h@Wh on top (start=False) and runs the
    per-step nonlinear chain on ACT/DVE/GPSIMD.
"""

import numpy as np

B_FULL = 64
B_LOC = 8
N_CORES = 8
S = 4096
E, C, NN = 64, 32, 16
EMB, HS, EF, DIM = 128, 256, 128, 64
G4 = 4 * HS
MC = 64              # steps per micro-chunk (gates PSUM capacity)
P = 128

# block order (f,i,g,o) x (half0, half1) -> column offset into the
# [i f g o] gate layout of Wx/Wh/bias
BLK_COL = [HS + 0, HS + 128, 0, 128, 2 * HS, 2 * HS + 128, 3 * HS, 3 * HS + 128]
# peephole weight row per block: f->Wc[1], i->Wc[0], g->none, o->Wc[2]
BLK_WC = [1, 1, 0, 0, None, None, 2, 2]


def build_nc(s_total=S, mc=MC):
    import concourse.bass as bass
    import concourse.tile as tile
    import concourse.mybir as mybir
    from concourse import bacc
    from concourse.bass import ds

    fp32 = mybir.dt.float32
    AF = mybir.ActivationFunctionType
    OP = mybir.AluOpType

    n_chunks = s_total // mc
    NCH_COLS = mc * B_LOC          # 512 cols per chunk (t-major, b-minor)

    nc = bacc.Bacc()

    event_d = nc.declare_dram_parameter("event", [B_LOC, s_total, E], fp32, isOutput=False)
    vc_d = nc.declare_dram_parameter("vc", [B_LOC, s_total, C], fp32, isOutput=False)
    vn_d = nc.declare_dram_parameter("vn", [B_LOC, s_total, NN], fp32, isOutput=False)
    h0_d = nc.declare_dram_parameter("h0", [B_LOC, HS], fp32, isOutput=False)
    c0_d = nc.declare_dram_parameter("c0", [B_LOC, HS], fp32, isOutput=False)
    Wx_d = nc.declare_dram_parameter("Wx", [EMB, G4], fp32, isOutput=False)
    Wh_d = nc.declare_dram_parameter("Wh", [HS, G4], fp32, isOutput=False)
    Wc_d = nc.declare_dram_parameter("Wc", [3, HS], fp32, isOutput=False)
    bias_d = nc.declare_dram_parameter("bias", [G4], fp32, isOutput=False)
    Ve_d = nc.declare_dram_parameter("Ve", [E, EMB], fp32, isOutput=False)
    Vc_d = nc.declare_dram_parameter("Vc", [C, EMB], fp32, isOutput=False)
    Vn_d = nc.declare_dram_parameter("Vn", [NN, EMB], fp32, isOutput=False)
    Wlin_d = nc.declare_dram_parameter("Wlin", [HS, DIM], fp32, isOutput=False)
    blin_d = nc.declare_dram_parameter("blin", [DIM], fp32, isOutput=False)
    Wef1_d = nc.declare_dram_parameter("Wef1", [EMB, EF], fp32, isOutput=False)
    bef1_d = nc.declare_dram_parameter("bef1", [EF], fp32, isOutput=False)
    Wef3_d = nc.declare_dram_parameter("Wef3", [EF, HS], fp32, isOutput=False)
    bef3_d = nc.declare_dram_parameter("bef3", [HS], fp32, isOutput=False)
    out_d = nc.declare_dram_parameter("out", [B_LOC, DIM], fp32, isOutput=True)

    with tile.TileContext(nc) as tc:
        with (
            tc.tile_pool(name="wts", bufs=1) as wts,
            tc.tile_pool(name="state", bufs=1) as stp,
            tc.tile_pool(name="chunk", bufs=2) as chp,
            tc.tile_pool(name="scr", bufs=3) as scr,
            tc.tile_pool(name="psum", bufs=1, space="PSUM") as psp,
        ):
            # ---------------- weights / constants into SBUF ----------------
            Wh_sb = wts.tile([P, 2, G4], fp32)       # [p, k, g]
            nc.sync.dma_start(Wh_sb[:], Wh_d.rearrange("(k p) g -> p k g", p=P))
            Wx_sb = wts.tile([P, G4], fp32)
            nc.sync.dma_start(Wx_sb[:], Wx_d[:])
            Ve_sb = wts.tile([E, EMB], fp32)
            nc.sync.dma_start(Ve_sb[:], Ve_d[:])
            Vc_sb = wts.tile([C, EMB], fp32)
            nc.sync.dma_start(Vc_sb[:], Vc_d[:])
            Vn_sb = wts.tile([NN, EMB], fp32)
            nc.sync.dma_start(Vn_sb[:], Vn_d[:])
            Wef1_sb = wts.tile([P, EF], fp32)
            nc.sync.dma_start(Wef1_sb[:], Wef1_d[:])
            Wef3_sb = wts.tile([P, HS], fp32)
            nc.sync.dma_start(Wef3_sb[:], Wef3_d[:])
            Wlin_sb = wts.tile([P, 2, DIM], fp32)
            nc.sync.dma_start(Wlin_sb[:], Wlin_d.rearrange("(k p) d -> p k d", p=P))
            brow_sb = wts.tile([1, G4], fp32)
            nc.sync.dma_start(brow_sb[:], bias_d.rearrange("(one g) -> one g", one=1))
            bef1_row = wts.tile([1, EF], fp32)
            nc.sync.dma_start(bef1_row[:], bef1_d.rearrange("(one g) -> one g", one=1))
            bef3_row = wts.tile([1, HS], fp32)
            nc.sync.dma_start(bef3_row[:], bef3_d.rearrange("(one g) -> one g", one=1))
            blin_col = wts.tile([DIM, 1], fp32)
            nc.sync.dma_start(blin_col[:], blin_d.rearrange("(d one) -> d one", one=1))
            ones_row = wts.tile([1, NCH_COLS], fp32)
            nc.vector.memset(ones_row[:], 1.0)

            # Vc scaled by 2 (x = s + 2*vc@Vc + 2*tanh(vn@Vn))
            Vc2_sb = wts.tile([C, EMB], fp32)
            nc.scalar.mul(Vc2_sb[:], Vc_sb[:], 2.0)

            # peephole weights broadcast: [p, blk, b]; g blocks zero
            wc_cols = wts.tile([P, 3, 2], fp32)      # [p, gate_idx, half]
            nc.sync.dma_start(wc_cols[:], Wc_d.rearrange("w (hf p) -> p w hf", p=P))
            ones8 = wts.tile([P, B_LOC], fp32)
            nc.vector.memset(ones8[:], 1.0)
            wcbc = wts.tile([P, 8, B_LOC], fp32)
            nc.vector.memset(wcbc[:], 0.0)
            for blk in range(8):
                gi = BLK_WC[blk]
                if gi is None:
                    continue
                hf = blk % 2
                nc.vector.tensor_scalar_mul(
                    wcbc[:, blk, :], ones8[:],
                    wc_cols[:, gi, hf : hf + 1],
                )

            # ---------------- state ----------------
            hT = stp.tile([P, 2, B_LOC], fp32)       # [p, half, b]
            # STATE = [c_hat(2,8) | c(2,8) | g(2,8)]
            STATE = stp.tile([P, 3, 2, B_LOC], fp32)
            for hf in range(2):
                nc.sync.dma_start(hT[:, hf, :],
                                  h0_d[:, hf * P:(hf + 1) * P].rearrange("b p -> p b"))
                nc.sync.dma_start(STATE[:, 1, hf, :],
                                  c0_d[:, hf * P:(hf + 1) * P].rearrange("b p -> p b"))

            # ---------------- main loop over micro-chunks ----------------
            def chunk_body(ci):
                t0 = ci * mc
                # -------- input DMAs (transposed loads) --------
                evT = chp.tile([E, mc, B_LOC], fp32, tag="evT")
                vcT = chp.tile([C, mc, B_LOC], fp32, tag="vcT")
                vnT = chp.tile([NN, mc, B_LOC], fp32, tag="vnT")
                for b in range(B_LOC):
                    nc.sync.dma_start(
                        evT[:, :, b], event_d[b, ds(t0, mc), :].rearrange("t e -> e t")
                    )
                    nc.sync.dma_start(
                        vcT[:, :, b], vc_d[b, ds(t0, mc), :].rearrange("t c -> c t")
                    )
                    nc.sync.dma_start(
                        vnT[:, :, b], vn_d[b, ds(t0, mc), :].rearrange("t n -> n t")
                    )

                banks = []
                for k in range(8):
                    bank_t = psp.tile([P, 8, 8, B_LOC], fp32, tag=f"bank{k}", name=f"bank{k}")  # [p, blk, t, b]
                    banks.append(bank_t)

                # -------- phase A: s, x, j for the whole chunk --------
                ps_x = banks[0][:].rearrange("p blk t b -> p (blk t b)")  # [128,512]
                ps_h = banks[1][:].rearrange("p blk t b -> p (blk t b)")
                # s = event @ Ve
                nc.tensor.matmul(ps_x, Ve_sb[:], evT[:].rearrange("e t b -> e (t b)"),
                                 start=True, stop=True)
                s_sb = chp.tile([P, NCH_COLS], fp32, tag="s_sb")
                nc.scalar.copy(s_sb[:], ps_x)
                # x = s + 2*vc@Vc + 2*tanh(vn@Vn)
                nc.tensor.matmul(ps_x, Vc2_sb[:], vcT[:].rearrange("c t b -> c (t b)"),
                                 start=False, stop=True, skip_group_check=True)
                nc.tensor.matmul(ps_h, Vn_sb[:], vnT[:].rearrange("n t b -> n (t b)"),
                                 start=True, stop=True)
                tn_sb = chp.tile([P, NCH_COLS], fp32, tag="tn_sb")
                nc.scalar.activation(tn_sb[:], ps_h, AF.Tanh)
                xT = chp.tile([P, mc, B_LOC], fp32, tag="xT")
                nc.vector.scalar_tensor_tensor(
                    xT[:].rearrange("p t b -> p (t b)"), tn_sb[:], 2.0, ps_x,
                    op0=OP.mult, op1=OP.add,
                )
                # u = tanh(s @ Wef1 + bef1)
                nc.tensor.matmul(ps_h, Wef1_sb[:], s_sb[:], start=True, stop=False)
                nc.tensor.matmul(ps_h, bef1_row[:], ones_row[:], start=False, stop=True)
                u_sb = chp.tile([P, NCH_COLS], fp32, tag="u_sb")
                nc.scalar.activation(u_sb[:], ps_h, AF.Tanh)
                # j = sigmoid(u @ Wef3 + bef3); jmj layout [p, t, (j0 j1 mj0 mj1), b]
                jmj = chp.tile([P, mc, 4, B_LOC], fp32, tag="jmj")
                for hf in range(2):
                    ps_j = banks[2 + hf][:].rearrange("p blk t b -> p (blk t b)")
                    nc.tensor.matmul(ps_j, Wef3_sb[:, hf * P : (hf + 1) * P], u_sb[:],
                                     start=True, stop=False)
                    nc.tensor.matmul(ps_j, bef3_row[:, hf * P : (hf + 1) * P],
                                     ones_row[:], start=False, stop=True)
                    nc.scalar.activation(jmj[:, :, hf, :], ps_j, AF.Sigmoid)
                # mj = 1 - j
                nc.scalar.activation(jmj[:, :, 2:4, :], jmj[:, :, 0:2, :],
                                     AF.Identity, bias=1.0, scale=-1.0)

                # -------- phase B: bias + x@Wx pre-accumulated into gates --------
                for blk in range(8):
                    co = BLK_COL[blk]
                    for k in range(8):
                        nc.tensor.matmul(
                            banks[k][:, blk, :, :], brow_sb[:, co : co + P],
                            ones_row[:, 0 : 8 * B_LOC],
                            start=(blk == 0), stop=False, skip_group_check=True,
                        )
                for blk in range(8):
                    co = BLK_COL[blk]
                    for k in range(8):
                        nc.tensor.matmul(
                            banks[k][:, blk, :, :], Wx_sb[:, co : co + P],
                            xT[:, 8 * k : 8 * k + 8, :],
                            start=False, stop=False, skip_group_check=True,
                        )

                # -------- phase C: the scan --------
                for tl in range(mc):
                    bk = banks[tl // 8]
                    trow = tl % 8
                    jmj_t = jmj[:, tl, :, :]

                    # peephole term cw = [c,c,c,c,0,0,c,c]*wcbc  (g rows of wcbc are 0)
                    cw = scr.tile([P, 4, 2, B_LOC], fp32, tag="cw")
                    nc.gpsimd.tensor_mul(
                        cw[:],
                        STATE[:, 1, :, :].unsqueeze(1).to_broadcast([P, 4, 2, B_LOC]),
                        wcbc[:].rearrange("p (r hf) b -> p r hf b", r=4),
                    )
                    # m2 = (1-j) * h   (independent of this step's gates)
                    m2T = scr.tile([P, 2, B_LOC], fp32, tag="m2T")
                    nc.gpsimd.tensor_mul(m2T[:], jmj_t[:, 2:4, :], hT[:])

                    # recurrent matmuls: g blocks first, then f,i, then o
                    order = [4, 5, 0, 1, 2, 3, 6, 7]
                    for n, blk in enumerate(order):
                        co = BLK_COL[blk]
                        for k in range(2):
                            nc.tensor.matmul(
                                bk[:, blk, trow, :], Wh_sb[:, k, co : co + P],
                                hT[:, k, :],
                                start=False, stop=(n == 7 and k == 1),
                                skip_group_check=True,
                            )

                    # pre-activations = gates + cw
                    pre = scr.tile([P, 8, B_LOC], fp32, tag="pre")
                    nc.vector.tensor_add(pre[:], bk[:, :, trow, :], cw[:].rearrange("p r hf b -> p (r hf) b"))
                    # activations
                    fi = scr.tile([P, 4, B_LOC], fp32, tag="fi")
                    nc.scalar.activation(fi[:], pre[:, 0:4, :], AF.Sigmoid)
                    nc.scalar.activation(STATE[:, 2, :, :], pre[:, 4:6, :], AF.Tanh)
                    oT = scr.tile([P, 2, B_LOC], fp32, tag="oT")
                    nc.scalar.activation(oT[:], pre[:, 6:8, :], AF.Sigmoid)
                    # c_hat = f*c + i*g
                    fcig = scr.tile([P, 4, B_LOC], fp32, tag="fcig")
                    nc.vector.tensor_mul(fcig[:], fi[:], STATE[:, 1:3, :, :].rearrange("p s hf b -> p (s hf) b"))
                    nc.vector.tensor_add(STATE[:, 0, :, :], fcig[:, 0:2, :], fcig[:, 2:4, :])
                    # c_new = j*c_hat + (1-j)*c
                    jcmj = scr.tile([P, 4, B_LOC], fp32, tag="jcmj")
                    nc.gpsimd.tensor_mul(jcmj[:], jmj_t[:], STATE[:, 0:2, :, :].rearrange("p s hf b -> p (s hf) b"))
                    nc.gpsimd.tensor_add(STATE[:, 1, :, :], jcmj[:, 0:2, :], jcmj[:, 2:4, :])
                    # h_new = j*o*tanh(c_hat) + (1-j)*h
                    thT = scr.tile([P, 2, B_LOC], fp32, tag="thT")
                    nc.scalar.activation(thT[:], STATE[:, 0, :, :], AF.Tanh)
                    joT = scr.tile([P, 2, B_LOC], fp32, tag="joT")
                    nc.gpsimd.tensor_mul(joT[:], jmj_t[:, 0:2, :], oT[:])
                    m1T = scr.tile([P, 2, B_LOC], fp32, tag="m1T")
                    nc.vector.tensor_mul(m1T[:], joT[:], thT[:])
                    nc.vector.tensor_add(hT[:], m1T[:], m2T[:])

            if n_chunks > 1:
                with tc.For_i(0, n_chunks, 1,
                              hint_engines=(mybir.EngineType.PE,
                                            mybir.EngineType.Activation,
                                            mybir.EngineType.DVE,
                                            mybir.EngineType.Pool)) as ci:
                    chunk_body(ci)
            else:
                chunk_body(0)

            # ---------------- output projection ----------------
            ps_o = psp.tile([DIM, B_LOC], fp32, tag="bank0")
            for k in range(2):
                nc.tensor.matmul(ps_o[:], Wlin_sb[:, k, :], hT[:, k, :],
                                 start=(k == 0), stop=(k == 1))
            outT = stp.tile([DIM, B_LOC], fp32)
            nc.scalar.activation(outT[:], ps_o[:], AF.Identity, bias=blin_col[:, 0:1])
            nc.sync.dma_start(out_d.rearrange("b d -> d b"), outT[:])

    nc.finalize()
    return nc


_NC_CACHE = {}


def _get_nc(s_total=S, mc=MC):
    key = (s_total, mc)
    if key not in _NC_CACHE:
        _NC_CACHE[key] = build_nc(s_total, mc)
    return _NC_CACHE[key]


def _make_in_maps(inputs, s_total=S):
    per_core = []
    w_names = ["Wx", "Wh", "Wc", "bias", "Ve", "Vc", "Vn", "Wlin", "blin",
               "Wef1", "bef1", "Wef3", "bef3"]
    for i in range(N_CORES):
        sl = slice(i * B_LOC, (i + 1) * B_LOC)
        m = {
            "event": np.ascontiguousarray(inputs["event"][sl, :s_total], np.float32),
            "vc": np.ascontiguousarray(inputs["vc"][sl, :s_total], np.float32),
            "vn": np.ascontiguousarray(inputs["vn"][sl, :s_total], np.float32),
            "h0": np.ascontiguousarray(inputs["h0"][sl], np.float32),
            "c0": np.ascontiguousarray(inputs["c0"][sl], np.float32),
        }
        for w in w_names:
            m[w] = np.ascontiguousarray(inputs[w], np.float32)
        per_core.append(m)
    return per_core


def run(inputs, s_total=S, mc=MC, trace=False):
    """Returns (out [B_FULL, DIM], exec_time_ns or None)."""
    from concourse.bass_utils import run_bass_kernel_spmd

    nc = _get_nc(s_total, mc)
    in_maps = _make_in_maps(inputs, s_total)
    res = run_bass_kernel_spmd(nc, in_maps, list(range(N_CORES)), trace=trace)
    out = np.concatenate([res.results[i]["out"] for i in range(N_CORES)], axis=0)
    return out, res.exec_time_ns


def kernel(**inputs):
    out, _ = run(inputs)
    return out



# revision 13
# speedup vs baseline: 3.4031x; 3.4031x over previous
"""HTSAD (event-filtered peephole LSTM) Trainium2 kernel, v2.

Data-parallel over batch (B=64 -> 8 cores x B_LOC=8); sequential scan over
S=4096 per core.

Key structure (vs the fp32 baseline):
  - All recurrent matmul weights (Wh, Wx, peephole diag(Wc), Wlin) are bf16
    -> LDWEIGHTS runs with fast-weight-load, ~4x cheaper than fp32.
  - One-tanh gate trick: the i/f/o columns of Wx/Wh/bias/Wc are pre-scaled
    by 0.5 host-side, so sigma(z) = (tanh(z/2)+1)/2 lets a single Tanh
    activation produce all four gates; the "+1" folds into the downstream
    scalar_tensor_tensor ops.
  - The peephole term c*Wc is accumulated into the gate PSUM by six
    diagonal bf16 matmuls (off the critical chain) instead of a DVE add.
  - Gates PSUM is laid out one gate-block per 256-column region
    [128p, 8blk, mc, 8b]; phase B (bias + x@Wx) is 16 big matmuls/chunk.
  - mc=32 chunks, ping-ponged across two 4-bank PSUM sets so the next
    chunk's batched phase (projections, j-gate, x@Wx) hides under the
    current chunk's scan.

Per-step critical chain:
  12 ifg h-matmuls -> tanh(ifg blocks) -> A=(t_if+1)*[g,c] -> ch2=A0+A1
  -> th=tanh(ch2/2) -> m1=jo*th -> h=m1+v
with v=(1-j)*h_prev, jo=(t_o+1)*(j/2), c'=(1-j)*c+(j/2)*ch2 computed
off-chain on Pool/DVE while the PE runs.
"""

import numpy as np

B_FULL = 64
B_LOC = 8
N_CORES = 8
S = 4096
E, C, NN = 64, 32, 16
EMB, HS, EF, DIM = 128, 256, 128, 64
G4 = 4 * HS
MC = 32               # steps per chunk (4 PSUM banks per chunk, x2 sets)
P = 128

# block order [i0,i1,f0,f1,g0,g1,o0,o1] -> column offset into [i f g o] gates
BLK_COL = [0, 128, 256, 384, 512, 640, 768, 896]
# peephole diag blocks (i,f,o x halves) and their Wc rows
DIAG_BLKS = [0, 1, 2, 3, 6, 7]
DIAG_SRC = [(0, 0), (0, 1), (1, 0), (1, 1), (2, 0), (2, 1)]


def build_nc(s_total=S, mc=MC, dbg_steps=None):
    import concourse.bass as bass
    import concourse.tile as tile
    import concourse.mybir as mybir
    from concourse import bacc
    from concourse.bass import ds

    fp32 = mybir.dt.float32
    bf16 = mybir.dt.bfloat16
    AF = mybir.ActivationFunctionType
    OP = mybir.AluOpType

    n_chunks = s_total // mc
    assert n_chunks % 2 == 0 and n_chunks >= 2
    n_body = n_chunks // 2 - 1
    NC = mc * B_LOC          # free columns per chunk-phase op (256)

    nc = bacc.Bacc()

    event_d = nc.declare_dram_parameter("event", [B_LOC, s_total, E], fp32, isOutput=False)
    vc_d = nc.declare_dram_parameter("vc", [B_LOC, s_total, C], fp32, isOutput=False)
    vn_d = nc.declare_dram_parameter("vn", [B_LOC, s_total, NN], fp32, isOutput=False)
    h0T_d = nc.declare_dram_parameter("h0T", [P, 2, B_LOC], bf16, isOutput=False)
    c0T_d = nc.declare_dram_parameter("c0T", [P, 2, B_LOC], bf16, isOutput=False)
    Whs_d = nc.declare_dram_parameter("Whs", [P, 2, G4], bf16, isOutput=False)
    Wxs_d = nc.declare_dram_parameter("Wxs", [P, G4], fp32, isOutput=False)
    biass_d = nc.declare_dram_parameter("biass", [1, G4], bf16, isOutput=False)
    dWc_d = nc.declare_dram_parameter("dWc", [P, 6, P], bf16, isOutput=False)
    Ve_d = nc.declare_dram_parameter("Ve", [E, EMB], fp32, isOutput=False)
    Vc2_d = nc.declare_dram_parameter("Vc2", [C, EMB], fp32, isOutput=False)
    Vn_d = nc.declare_dram_parameter("Vn", [NN, EMB], fp32, isOutput=False)
    Wef1_d = nc.declare_dram_parameter("Wef1", [EMB, EF], fp32, isOutput=False)
    Wef3_d = nc.declare_dram_parameter("Wef3", [EF, HS], fp32, isOutput=False)
    bef1c_d = nc.declare_dram_parameter("bef1c", [EF, 1], fp32, isOutput=False)
    nbef3_d = nc.declare_dram_parameter("nbef3", [P, 2], fp32, isOutput=False)
    Wlins_d = nc.declare_dram_parameter("Wlins", [P, 2, DIM], bf16, isOutput=False)
    blinc_d = nc.declare_dram_parameter("blinc", [DIM, 1], fp32, isOutput=False)
    out_d = nc.declare_dram_parameter("out", [B_LOC, DIM], fp32, isOutput=True)
    if dbg_steps is not None:
        ts_dbg_d = nc.declare_dram_parameter("ts_dbg", [P, 12, B_LOC], fp32, isOutput=True)
        h_dbg_d = nc.declare_dram_parameter("h_dbg", [P, 2, B_LOC], fp32, isOutput=True)
        x_dbg_d = nc.declare_dram_parameter("x_dbg", [P, mc, B_LOC], fp32, isOutput=True)
        j_dbg_d = nc.declare_dram_parameter("j_dbg", [P, mc, 4, B_LOC], fp32, isOutput=True)
        g_dbg_d = nc.declare_dram_parameter("g_dbg", [P, 8, B_LOC], fp32, isOutput=True)

    with tile.TileContext(nc) as tc:
        with (
            tc.tile_pool(name="wts", bufs=1) as wts,
            tc.tile_pool(name="stp", bufs=1) as stp,
            tc.tile_pool(name="io", bufs=1) as io,
            tc.tile_pool(name="ph", bufs=2) as ph,
            tc.tile_pool(name="scr", bufs=3) as scr,
            tc.tile_pool(name="psum", bufs=1, space="PSUM") as psp,
        ):
            # ---------------- weights into SBUF ----------------
            Whs_sb = wts.tile([P, 2, G4], bf16)
            nc.sync.dma_start(Whs_sb[:], Whs_d[:])
            Wxs_sb = wts.tile([P, G4], fp32)
            nc.sync.dma_start(Wxs_sb[:], Wxs_d[:])
            biass_sb = wts.tile([1, G4], bf16)
            nc.sync.dma_start(biass_sb[:], biass_d[:])
            dWc_sb = wts.tile([P, 6, P], bf16)
            nc.sync.dma_start(dWc_sb[:], dWc_d[:])
            Ve_sb = wts.tile([E, EMB], fp32)
            nc.sync.dma_start(Ve_sb[:], Ve_d[:])
            Vc2_sb = wts.tile([C, EMB], fp32)
            nc.sync.dma_start(Vc2_sb[:], Vc2_d[:])
            Vn_sb = wts.tile([NN, EMB], fp32)
            nc.sync.dma_start(Vn_sb[:], Vn_d[:])
            Wef1_sb = wts.tile([EMB, EF], fp32)
            nc.sync.dma_start(Wef1_sb[:], Wef1_d[:])
            Wef3_sb = wts.tile([EF, HS], fp32)
            nc.sync.dma_start(Wef3_sb[:], Wef3_d[:])
            bef1c_sb = wts.tile([EF, 1], fp32)
            nc.sync.dma_start(bef1c_sb[:], bef1c_d[:])
            nbef3_sb = wts.tile([P, 2], fp32)
            nc.sync.dma_start(nbef3_sb[:], nbef3_d[:])
            Wlins_sb = wts.tile([P, 2, DIM], bf16)
            nc.sync.dma_start(Wlins_sb[:], Wlins_d[:])
            blinc_sb = wts.tile([DIM, 1], fp32)
            nc.sync.dma_start(blinc_sb[:], blinc_d[:])
            ones_bf = wts.tile([1, 2 * NC], bf16)
            nc.vector.memset(ones_bf[:], 1.0)
            zrow = wts.tile([1, P], bf16)
            nc.vector.memset(zrow[:], 0.0)

            # ---------------- persistent state ----------------
            # TS blocks: [t_i(0:2) t_f(2:4) t_g(4:6) c(6:8) ch2(8:10) t_o(10:12)]
            TS = stp.tile([P, 12, B_LOC], bf16)
            hT = stp.tile([P, 2, B_LOC], bf16)
            nc.sync.dma_start(TS[:, 6:8, :], c0T_d[:])
            nc.sync.dma_start(hT[:], h0T_d[:])

            # ---------------- per-parity chunk resources ----------------
            gts, evT, vcT, vnT, xT, jmj = [], [], [], [], [], []
            for par in range(2):
                gts.append(psp.tile([P, 8, mc, B_LOC], fp32, tag=f"g{par}",
                                    name=f"g{par}"))
                evT.append(io.tile([E, mc, B_LOC], fp32, tag=f"ev{par}",
                                   name=f"ev{par}"))
                vcT.append(io.tile([C, mc, B_LOC], fp32, tag=f"vcT{par}",
                                   name=f"vcT{par}"))
                vnT.append(io.tile([NN, mc, B_LOC], fp32, tag=f"vnT{par}",
                                   name=f"vnT{par}"))
                xT.append(io.tile([P, mc, B_LOC], fp32, tag=f"x{par}",
                                  name=f"x{par}"))
                # jmj blocks: [mj0 mj1 jh0 jh1]
                jmj.append(io.tile([P, mc, 4, B_LOC], bf16, tag=f"jmj{par}",
                                   name=f"jmj{par}"))

            def phases(t0, par):
                """Batched work for the chunk starting at step t0 (parity par):
                input DMAs, projections, j-gate, and bias + x@Wx into PSUM."""
                g = gts[par]
                gv = g[:].rearrange("p blk t b -> p (blk t b)")
                for b in range(B_LOC):
                    nc.sync.dma_start(
                        evT[par][:, :, b],
                        event_d[b, ds(t0, mc), :].rearrange("t e -> e t"))
                    nc.sync.dma_start(
                        vcT[par][:, :, b],
                        vc_d[b, ds(t0, mc), :].rearrange("t c -> c t"))
                    nc.sync.dma_start(
                        vnT[par][:, :, b],
                        vn_d[b, ds(t0, mc), :].rearrange("t n -> n t"))

                # scratch regions on distinct banks (start=True clears the
                # WHOLE bank's has_written bits, so each region that still
                # accumulates must not share a bank with a later start)
                R0 = gv[:, 0:NC]                    # block0, bank0
                Rvn = gv[:, 2 * NC:3 * NC]          # block2, bank1
                Ru = gv[:, 4 * NC:5 * NC]           # block4, bank2
                Rj0 = gv[:, 6 * NC:7 * NC]          # block6, bank3
                Rj1 = gv[:, 0:NC]                   # block0 again (after xTb)
                s_sb = ph.tile([P, NC], fp32, tag="s_sb")
                tn_sb = ph.tile([P, NC], fp32, tag="tn_sb")
                u_sb = ph.tile([P, NC], fp32, tag="u_sb")

                # s = event @ Ve ; x = s + vc@Vc2 + 2*tanh(vn@Vn)
                nc.tensor.matmul(R0, Ve_sb[:], evT[par][:].rearrange("e t b -> e (t b)"),
                                 start=True, stop=False, skip_group_check=True)
                nc.vector.tensor_scalar_add(s_sb[:], R0, 0.0)
                nc.tensor.matmul(R0, Vc2_sb[:], vcT[par][:].rearrange("c t b -> c (t b)"),
                                 start=False, stop=True, skip_group_check=True)
                nc.tensor.matmul(Rvn, Vn_sb[:], vnT[par][:].rearrange("n t b -> n (t b)"),
                                 start=True, stop=True, skip_group_check=True)
                nc.scalar.activation(tn_sb[:], Rvn, AF.Tanh)
                nc.vector.scalar_tensor_tensor(
                    xT[par][:].rearrange("p t b -> p (t b)"), tn_sb[:], 2.0, R0,
                    op0=OP.mult, op1=OP.add)
                # u = tanh(s @ Wef1 + bef1)
                nc.tensor.matmul(Ru, Wef1_sb[:], s_sb[:],
                                 start=True, stop=True, skip_group_check=True)
                nc.scalar.activation(u_sb[:], Ru, AF.Tanh, bias=bef1c_sb[:, 0:1])
                # mj = 1 - sigmoid(u @ Wef3 + bef3) = sigmoid(-z - bef3)
                nc.tensor.matmul(Rj0, Wef3_sb[:, 0:P], u_sb[:],
                                 start=True, stop=True, skip_group_check=True)
                nc.scalar.activation(jmj[par][:, :, 0, :],
                                     Rj0.rearrange("p (t b) -> p t b", b=B_LOC),
                                     AF.Sigmoid, scale=-1.0, bias=nbef3_sb[:, 0:1])
                nc.tensor.matmul(Rj1, Wef3_sb[:, P:2 * P], u_sb[:],
                                 start=True, stop=True, skip_group_check=True)
                nc.scalar.activation(jmj[par][:, :, 1, :],
                                     Rj1.rearrange("p (t b) -> p t b", b=B_LOC),
                                     AF.Sigmoid, scale=-1.0, bias=nbef3_sb[:, 1:2])
                # jh = j/2 = -0.5*mj + 0.5
                nc.vector.tensor_scalar(jmj[par][:, :, 2, :], jmj[par][:, :, 0, :],
                                        -0.5, 0.5, op0=OP.mult, op1=OP.add)
                nc.vector.tensor_scalar(jmj[par][:, :, 3, :], jmj[par][:, :, 1, :],
                                        -0.5, 0.5, op0=OP.mult, op1=OP.add)

                # phase B: per bank, ONE bank-wide start=True zeroing matmul,
                # then bias + x@Wx accumulate with start=False.
                xf = xT[par][:].rearrange("p t b -> p (t b)")
                for bk in range(4):
                    nc.tensor.matmul(
                        g[:, 2 * bk:2 * bk + 2, :, :], zrow[:], ones_bf[:],
                        start=True, stop=False, skip_group_check=True)
                for blk in range(8):
                    co = BLK_COL[blk]
                    nc.tensor.matmul(g[:, blk, :, :], biass_sb[:, co:co + P],
                                     ones_bf[:, 0:NC], start=False, stop=False,
                                     skip_group_check=True)
                    nc.tensor.matmul(g[:, blk, :, :], Wxs_sb[:, co:co + P], xf,
                                     start=False, stop=False, skip_group_check=True)

            def scan(par, nsteps=mc):
                """Sequential scan over one chunk's mc steps."""
                g = gts[par]
                jm = jmj[par]
                for t in range(nsteps):
                    # v = (1-j) * h_{t-1}  (off-chain, reads old h)
                    v_t = scr.tile([P, 2, B_LOC], bf16, tag="v")
                    nc.gpsimd.tensor_mul(v_t[:], jm[:, t, 0:2, :], hT[:])
                    # peephole: gates_ifo += c * Wc/2 (diag matmuls, off-chain)
                    for d, blk in enumerate(DIAG_BLKS):
                        hf = d % 2
                        nc.tensor.matmul(g[:, blk, t, :], dWc_sb[:, d, :],
                                         TS[:, 6 + hf:7 + hf, :],
                                         start=False, stop=False,
                                         skip_group_check=True)
                    # recurrent matmuls: i,f,g blocks first (critical), o last
                    for blk in (0, 1, 2, 3, 4, 5, 6, 7):
                        co = BLK_COL[blk]
                        for k in range(2):
                            nc.tensor.matmul(g[:, blk, t, :],
                                             Whs_sb[:, k, co:co + P], hT[:, k, :],
                                             start=False, stop=(k == 1),
                                             skip_group_check=True)
                    # all gates via one function: t = tanh(bank)
                    nc.scalar.activation(TS[:, 0:6, :], g[:, 0:6, t, :], AF.Tanh)
                    nc.scalar.activation(TS[:, 10:12, :], g[:, 6:8, t, :], AF.Tanh)
                    # A = (t_if + 1) * [g, c] ; ch2 = 2*c_hat = A[0:2] + A[2:4]
                    A_t = scr.tile([P, 4, B_LOC], bf16, tag="A")
                    nc.vector.scalar_tensor_tensor(A_t[:], TS[:, 0:4, :], 1.0,
                                                   TS[:, 4:8, :],
                                                   op0=OP.add, op1=OP.mult)
                    nc.vector.tensor_add(TS[:, 8:10, :], A_t[:, 0:2, :], A_t[:, 2:4, :])
                    # th = tanh(c_hat) = tanh(0.5 * ch2)
                    th_t = scr.tile([P, 2, B_LOC], bf16, tag="th")
                    nc.scalar.activation(th_t[:], TS[:, 8:10, :], AF.Tanh, scale=0.5)
                    # jo = (t_o + 1) * jh = j*o  (off-chain)
                    jo_t = scr.tile([P, 2, B_LOC], bf16, tag="jo")
                    nc.vector.scalar_tensor_tensor(jo_t[:], TS[:, 10:12, :], 1.0,
                                                   jm[:, t, 2:4, :],
                                                   op0=OP.add, op1=OP.mult)
                    # c' = mj*c + jh*ch2  (off-chain)
                    ccm_t = scr.tile([P, 4, B_LOC], bf16, tag="ccm")
                    nc.gpsimd.tensor_mul(ccm_t[:], jm[:, t, :, :], TS[:, 6:10, :])
                    nc.gpsimd.tensor_add(TS[:, 6:8, :], ccm_t[:, 0:2, :], ccm_t[:, 2:4, :])
                    # h = jo*th + v
                    m1_t = scr.tile([P, 2, B_LOC], bf16, tag="m1")
                    nc.vector.tensor_mul(m1_t[:], jo_t[:], th_t[:])
                    nc.vector.tensor_add(hT[:], m1_t[:], v_t[:])

            # ---------------- prologue ----------------
            phases(0, 0)
            phases(mc, 1)

            if dbg_steps is not None:
                scan(0, dbg_steps)
                stg = stp.tile([P, 12, B_LOC], fp32, name="stg")
                nc.vector.tensor_scalar_add(stg[:], TS[:], 0.0)
                nc.sync.dma_start(ts_dbg_d[:], stg[:])
                stg2 = stp.tile([P, 2, B_LOC], fp32, name="stg2")
                nc.vector.tensor_scalar_add(stg2[:], hT[:], 0.0)
                nc.sync.dma_start(h_dbg_d[:], stg2[:])
                stg3 = stp.tile([P, mc, B_LOC], fp32, name="stg3")
                nc.vector.tensor_scalar_add(stg3[:], xT[0][:], 0.0)
                nc.sync.dma_start(x_dbg_d[:], stg3[:])
                stg4 = stp.tile([P, mc, 4, B_LOC], fp32, name="stg4")
                nc.vector.tensor_scalar_add(stg4[:], jmj[0][:], 0.0)
                nc.sync.dma_start(j_dbg_d[:], stg4[:])
                stg5 = stp.tile([P, 8, B_LOC], fp32, name="stg5")
                nc.vector.tensor_scalar_add(stg5[:], gts[0][:, :, max(dbg_steps - 1, 0), :], 0.0)
                nc.sync.dma_start(g_dbg_d[:], stg5[:])

            if dbg_steps is None:
                # ---------------- main loop ----------------
                def body(i):
                    scan(0)                       # chunk 2i
                    phases(i * (2 * mc) + 2 * mc, 0)      # chunk 2i+2
                    scan(1)                       # chunk 2i+1
                    phases(i * (2 * mc) + 3 * mc, 1)      # chunk 2i+3

                if n_body > 0:
                    with tc.For_i(0, n_body, 1,
                                  hint_engines=(mybir.EngineType.PE,
                                                mybir.EngineType.Activation,
                                                mybir.EngineType.DVE,
                                                mybir.EngineType.Pool)) as i:
                        body(i)

                # ---------------- epilogue: last two chunks ----------------
                scan(0)
                scan(1)

                # ---------------- output projection ----------------
                ps_o = psp.tile([DIM, B_LOC], fp32, tag="g0")
                for k in range(2):
                    nc.tensor.matmul(ps_o[:], Wlins_sb[:, k, :], hT[:, k, :],
                                     start=(k == 0), stop=(k == 1),
                                     skip_group_check=True)
                outT = stp.tile([DIM, B_LOC], fp32)
                nc.scalar.activation(outT[:], ps_o[:], AF.Identity, bias=blinc_sb[:, 0:1])
                nc.sync.dma_start(out_d.rearrange("b d -> d b"), outT[:])

    nc.finalize()
    return nc


_NC_CACHE = {}


def _get_nc(s_total=S, mc=MC, dbg_steps=None):
    key = (s_total, mc, dbg_steps)
    if key not in _NC_CACHE:
        _NC_CACHE[key] = build_nc(s_total, mc, dbg_steps)
    return _NC_CACHE[key]


def run_dbg(inputs, s_total, dbg_steps):
    from concourse.bass_utils import run_bass_kernel_spmd

    nc = _get_nc(s_total, MC, dbg_steps)
    in_maps = _make_in_maps(inputs, s_total)
    res = run_bass_kernel_spmd(nc, in_maps[:1], [0])
    return res.results[0]


def _prep_shared(inputs):
    import concourse.mybir as mybir
    bf = mybir.dt.np(mybir.dt.bfloat16)

    colscale = np.full((G4,), 0.5, np.float32)
    colscale[2 * HS:3 * HS] = 1.0          # g columns unscaled

    Wh = np.asarray(inputs["Wh"], np.float32) * colscale
    Whs = np.ascontiguousarray(Wh.reshape(2, P, G4).transpose(1, 0, 2)).astype(bf)
    Wxs = np.ascontiguousarray(np.asarray(inputs["Wx"], np.float32) * colscale)
    biass = (np.asarray(inputs["bias"], np.float32) * colscale).reshape(1, G4).astype(bf)

    Wc = np.asarray(inputs["Wc"], np.float32)
    dWc = np.zeros((P, 6, P), np.float32)
    for d, (gi, hf) in enumerate(DIAG_SRC):
        seg = Wc[gi, hf * P:(hf + 1) * P] * 0.5
        dWc[np.arange(P), d, np.arange(P)] = seg
    dWc = dWc.astype(bf)

    Wlin = np.asarray(inputs["Wlin"], np.float32)
    Wlins = np.ascontiguousarray(Wlin.reshape(2, P, DIM).transpose(1, 0, 2)).astype(bf)

    return {
        "Whs": Whs, "Wxs": Wxs, "biass": biass, "dWc": dWc, "Wlins": Wlins,
        "Ve": np.ascontiguousarray(inputs["Ve"], np.float32),
        "Vc2": np.ascontiguousarray(2.0 * np.asarray(inputs["Vc"], np.float32)),
        "Vn": np.ascontiguousarray(inputs["Vn"], np.float32),
        "Wef1": np.ascontiguousarray(inputs["Wef1"], np.float32),
        "Wef3": np.ascontiguousarray(inputs["Wef3"], np.float32),
        "bef1c": np.ascontiguousarray(np.asarray(inputs["bef1"], np.float32).reshape(EF, 1)),
        "nbef3": np.ascontiguousarray((-np.asarray(inputs["bef3"], np.float32)).reshape(2, P).T),
        "blinc": np.ascontiguousarray(np.asarray(inputs["blin"], np.float32).reshape(DIM, 1)),
    }


def _make_in_maps(inputs, s_total=S):
    import concourse.mybir as mybir
    bf = mybir.dt.np(mybir.dt.bfloat16)
    shared = _prep_shared(inputs)

    def stateT(x):   # [B_LOC, 256] -> [128, 2, B_LOC] bf16
        return np.ascontiguousarray(
            x.T.reshape(2, P, B_LOC).transpose(1, 0, 2)).astype(bf)

    per_core = []
    for i in range(N_CORES):
        sl = slice(i * B_LOC, (i + 1) * B_LOC)
        m = dict(shared)
        m["event"] = np.ascontiguousarray(inputs["event"][sl, :s_total], np.float32)
        m["vc"] = np.ascontiguousarray(inputs["vc"][sl, :s_total], np.float32)
        m["vn"] = np.ascontiguousarray(inputs["vn"][sl, :s_total], np.float32)
        m["h0T"] = stateT(np.asarray(inputs["h0"], np.float32)[sl])
        m["c0T"] = stateT(np.asarray(inputs["c0"], np.float32)[sl])
        per_core.append(m)
    return per_core


def run(inputs, s_total=S, mc=MC, trace=False):
    """Returns (out [B_FULL, DIM], exec_time_ns or None)."""
    from concourse.bass_utils import run_bass_kernel_spmd

    nc = _get_nc(s_total, mc)
    in_maps = _make_in_maps(inputs, s_total)
    res = run_bass_kernel_spmd(nc, in_maps, list(range(N_CORES)), trace=trace)
    out = np.concatenate([res.results[i]["out"] for i in range(N_CORES)], axis=0)
    return out, res.exec_time_ns


def kernel(**inputs):
    out, _ = run(inputs)
    return out


# revision 17
# speedup vs baseline: 3.6191x; 1.0635x over previous
"""HTSAD (event-filtered peephole LSTM) Trainium2 kernel, v2.

Data-parallel over batch (B=64 -> 8 cores x B_LOC=8); sequential scan over
S=4096 per core.

Key structure (vs the fp32 baseline):
  - All recurrent matmul weights (Wh, Wx, peephole diag(Wc), Wlin) are bf16
    -> LDWEIGHTS runs with fast-weight-load, ~4x cheaper than fp32.
  - One-tanh gate trick: the i/f/o columns of Wx/Wh/bias/Wc are pre-scaled
    by 0.5 host-side, so sigma(z) = (tanh(z/2)+1)/2 lets a single Tanh
    activation produce all four gates; the "+1" folds into the downstream
    scalar_tensor_tensor ops.
  - The peephole term c*Wc is accumulated into the gate PSUM by six
    diagonal bf16 matmuls (off the critical chain) instead of a DVE add.
  - Gates PSUM is laid out one gate-block per 256-column region
    [128p, 8blk, mc, 8b]; phase B (bias + x@Wx) is 16 big matmuls/chunk.
  - mc=32 chunks, ping-ponged across two 4-bank PSUM sets so the next
    chunk's batched phase (projections, j-gate, x@Wx) hides under the
    current chunk's scan.

Per-step critical chain:
  12 ifg h-matmuls -> tanh(ifg blocks) -> A=(t_if+1)*[g,c] -> ch2=A0+A1
  -> th=tanh(ch2/2) -> m1=jo*th -> h=m1+v
with v=(1-j)*h_prev, jo=(t_o+1)*(j/2), c'=(1-j)*c+(j/2)*ch2 computed
off-chain on Pool/DVE while the PE runs.
"""

import numpy as np

B_FULL = 64
B_LOC = 8
N_CORES = 8
S = 4096
E, C, NN = 64, 32, 16
EMB, HS, EF, DIM = 128, 256, 128, 64
G4 = 4 * HS
MC = 32               # steps per chunk (4 PSUM banks per chunk, x2 sets)
P = 128

# block order [i0,i1,f0,f1,g0,g1,o0,o1] -> column offset into [i f g o] gates
BLK_COL = [0, 128, 256, 384, 512, 640, 768, 896]
# peephole diag blocks (i,f,o x halves) and their Wc rows
DIAG_BLKS = [0, 1, 2, 3, 6, 7]
DIAG_SRC = [(0, 0), (0, 1), (1, 0), (1, 1), (2, 0), (2, 1)]


def build_nc(s_total=S, mc=MC, dbg_steps=None):
    import concourse.bass as bass
    import concourse.tile as tile
    import concourse.mybir as mybir
    from concourse import bacc
    from concourse.bass import ds

    fp32 = mybir.dt.float32
    bf16 = mybir.dt.bfloat16
    AF = mybir.ActivationFunctionType
    OP = mybir.AluOpType

    n_chunks = s_total // mc
    assert n_chunks % 2 == 0 and n_chunks >= 2
    n_body = n_chunks // 2 - 1
    NC = mc * B_LOC          # free columns per chunk-phase op (256)

    nc = bacc.Bacc()

    event_d = nc.declare_dram_parameter("event", [B_LOC, s_total, E], fp32, isOutput=False)
    vc_d = nc.declare_dram_parameter("vc", [B_LOC, s_total, C], fp32, isOutput=False)
    vn_d = nc.declare_dram_parameter("vn", [B_LOC, s_total, NN], fp32, isOutput=False)
    h0T_d = nc.declare_dram_parameter("h0T", [P, 2, B_LOC], bf16, isOutput=False)
    c0T_d = nc.declare_dram_parameter("c0T", [P, 2, B_LOC], bf16, isOutput=False)
    Whs_d = nc.declare_dram_parameter("Whs", [P, 2, G4], bf16, isOutput=False)
    Wxs_d = nc.declare_dram_parameter("Wxs", [P, G4], fp32, isOutput=False)
    biass_d = nc.declare_dram_parameter("biass", [1, G4], bf16, isOutput=False)
    dWc_d = nc.declare_dram_parameter("dWc", [P, 6, P], bf16, isOutput=False)
    Ve_d = nc.declare_dram_parameter("Ve", [E, EMB], fp32, isOutput=False)
    Vc2_d = nc.declare_dram_parameter("Vc2", [C, EMB], fp32, isOutput=False)
    Vn_d = nc.declare_dram_parameter("Vn", [NN, EMB], fp32, isOutput=False)
    Wef1_d = nc.declare_dram_parameter("Wef1", [EMB, EF], fp32, isOutput=False)
    Wef3_d = nc.declare_dram_parameter("Wef3", [EF, HS], fp32, isOutput=False)
    bef1c_d = nc.declare_dram_parameter("bef1c", [EF, 1], fp32, isOutput=False)
    nbef3_d = nc.declare_dram_parameter("nbef3", [P, 2], fp32, isOutput=False)
    Wlins_d = nc.declare_dram_parameter("Wlins", [P, 2, DIM], bf16, isOutput=False)
    blinc_d = nc.declare_dram_parameter("blinc", [DIM, 1], fp32, isOutput=False)
    out_d = nc.declare_dram_parameter("out", [B_LOC, DIM], fp32, isOutput=True)
    if dbg_steps is not None:
        ts_dbg_d = nc.declare_dram_parameter("ts_dbg", [P, 12, B_LOC], fp32, isOutput=True)
        h_dbg_d = nc.declare_dram_parameter("h_dbg", [P, 2, B_LOC], fp32, isOutput=True)
        x_dbg_d = nc.declare_dram_parameter("x_dbg", [P, mc, B_LOC], fp32, isOutput=True)
        j_dbg_d = nc.declare_dram_parameter("j_dbg", [P, mc, 4, B_LOC], fp32, isOutput=True)
        g_dbg_d = nc.declare_dram_parameter("g_dbg", [P, 8, B_LOC], fp32, isOutput=True)

    with tile.TileContext(nc) as tc:
        with (
            tc.tile_pool(name="wts", bufs=1) as wts,
            tc.tile_pool(name="stp", bufs=1) as stp,
            tc.tile_pool(name="io", bufs=1) as io,
            tc.tile_pool(name="ph", bufs=2) as ph,
            tc.tile_pool(name="scr", bufs=3) as scr,
            tc.tile_pool(name="psum", bufs=1, space="PSUM") as psp,
        ):
            # ---------------- weights into SBUF ----------------
            Whs_sb = wts.tile([P, 2, G4], bf16)
            nc.sync.dma_start(Whs_sb[:], Whs_d[:])
            Wxs_sb = wts.tile([P, G4], fp32)
            nc.sync.dma_start(Wxs_sb[:], Wxs_d[:])
            biass_sb = wts.tile([1, G4], bf16)
            nc.sync.dma_start(biass_sb[:], biass_d[:])
            dWc_sb = wts.tile([P, 6, P], bf16)
            nc.sync.dma_start(dWc_sb[:], dWc_d[:])
            Ve_sb = wts.tile([E, EMB], fp32)
            nc.sync.dma_start(Ve_sb[:], Ve_d[:])
            Vc2_sb = wts.tile([C, EMB], fp32)
            nc.sync.dma_start(Vc2_sb[:], Vc2_d[:])
            Vn_sb = wts.tile([NN, EMB], fp32)
            nc.sync.dma_start(Vn_sb[:], Vn_d[:])
            Wef1_sb = wts.tile([EMB, EF], fp32)
            nc.sync.dma_start(Wef1_sb[:], Wef1_d[:])
            Wef3_sb = wts.tile([EF, HS], fp32)
            nc.sync.dma_start(Wef3_sb[:], Wef3_d[:])
            bef1c_sb = wts.tile([EF, 1], fp32)
            nc.sync.dma_start(bef1c_sb[:], bef1c_d[:])
            nbef3_sb = wts.tile([P, 2], fp32)
            nc.sync.dma_start(nbef3_sb[:], nbef3_d[:])
            Wlins_sb = wts.tile([P, 2, DIM], bf16)
            nc.sync.dma_start(Wlins_sb[:], Wlins_d[:])
            blinc_sb = wts.tile([DIM, 1], fp32)
            nc.sync.dma_start(blinc_sb[:], blinc_d[:])
            ones_bf = wts.tile([1, 2 * NC], bf16)
            nc.vector.memset(ones_bf[:], 1.0)
            zrow = wts.tile([1, P], bf16)
            nc.vector.memset(zrow[:], 0.0)

            # ---------------- persistent state ----------------
            # TS blocks: [t_i(0:2) t_f(2:4) t_g(4:6) c(6:8) ch2(8:10) t_o(10:12)]
            TS = stp.tile([P, 12, B_LOC], bf16)
            hT = stp.tile([P, 2, B_LOC], bf16)
            nc.sync.dma_start(TS[:, 6:8, :], c0T_d[:])
            nc.sync.dma_start(hT[:], h0T_d[:])
            # MV holds [m1 | v]; h = m1 + v is accumulated inside the gate
            # matmuls (broadcast out AP). Seed with [h0 | 0].
            MV0 = stp.tile([P, 2, 2, B_LOC], bf16)
            nc.sync.dma_start(MV0[:, :, 0, :], h0T_d[:])
            nc.vector.memset(MV0[:, :, 1, :], 0.0)
            MV_prev = [MV0]

            # ---------------- per-parity chunk resources ----------------
            gts, evT, vcT, vnT, xT, jmj = [], [], [], [], [], []
            for par in range(2):
                gts.append(psp.tile([P, 8, mc, B_LOC], fp32, tag=f"g{par}",
                                    name=f"g{par}"))
                evT.append(io.tile([E, mc, B_LOC], fp32, tag=f"ev{par}",
                                   name=f"ev{par}"))
                vcT.append(io.tile([C, mc, B_LOC], fp32, tag=f"vcT{par}",
                                   name=f"vcT{par}"))
                vnT.append(io.tile([NN, mc, B_LOC], fp32, tag=f"vnT{par}",
                                   name=f"vnT{par}"))
                xT.append(io.tile([P, mc, B_LOC], fp32, tag=f"x{par}",
                                  name=f"x{par}"))
                # jmj blocks: [mj0 mj1 jh0 jh1]
                jmj.append(io.tile([P, mc, 4, B_LOC], bf16, tag=f"jmj{par}",
                                   name=f"jmj{par}"))

            def phases(t0, par):
                """Batched work for the chunk starting at step t0 (parity par):
                input DMAs, projections, j-gate, and bias + x@Wx into PSUM.
                Generator: yields between instruction groups so the caller can
                interleave emission into the scan's per-step stream."""
                g = gts[par]
                gv = g[:].rearrange("p blk t b -> p (blk t b)")
                for b in range(B_LOC):
                    nc.sync.dma_start(
                        evT[par][:, :, b],
                        event_d[b, ds(t0, mc), :].rearrange("t e -> e t"))
                    nc.sync.dma_start(
                        vcT[par][:, :, b],
                        vc_d[b, ds(t0, mc), :].rearrange("t c -> c t"))
                    nc.sync.dma_start(
                        vnT[par][:, :, b],
                        vn_d[b, ds(t0, mc), :].rearrange("t n -> n t"))
                    if b % 2 == 1:
                        yield

                # scratch regions on distinct banks (start=True clears the
                # WHOLE bank's has_written bits, so each region that still
                # accumulates must not share a bank with a later start)
                R0 = gv[:, 0:NC]                    # block0, bank0
                Rvn = gv[:, 2 * NC:3 * NC]          # block2, bank1
                Ru = gv[:, 4 * NC:5 * NC]           # block4, bank2
                Rj0 = gv[:, 6 * NC:7 * NC]          # block6, bank3
                Rj1 = gv[:, 0:NC]                   # block0 again (after xTb)
                s_sb = ph.tile([P, NC], fp32, tag="s_sb")
                tn_sb = ph.tile([P, NC], fp32, tag="tn_sb")
                u_sb = ph.tile([P, NC], fp32, tag="u_sb")

                # s = event @ Ve ; x = s + vc@Vc2 + 2*tanh(vn@Vn)
                nc.tensor.matmul(R0, Ve_sb[:], evT[par][:].rearrange("e t b -> e (t b)"),
                                 start=True, stop=False, skip_group_check=True)
                yield
                nc.vector.tensor_scalar_add(s_sb[:], R0, 0.0)
                yield
                nc.tensor.matmul(R0, Vc2_sb[:], vcT[par][:].rearrange("c t b -> c (t b)"),
                                 start=False, stop=True, skip_group_check=True)
                yield
                nc.tensor.matmul(Rvn, Vn_sb[:], vnT[par][:].rearrange("n t b -> n (t b)"),
                                 start=True, stop=True, skip_group_check=True)
                yield
                nc.scalar.activation(tn_sb[:], Rvn, AF.Tanh)
                yield
                nc.vector.scalar_tensor_tensor(
                    xT[par][:].rearrange("p t b -> p (t b)"), tn_sb[:], 2.0, R0,
                    op0=OP.mult, op1=OP.add)
                yield
                # u = tanh(s @ Wef1 + bef1)
                nc.tensor.matmul(Ru, Wef1_sb[:], s_sb[:],
                                 start=True, stop=True, skip_group_check=True)
                yield
                nc.scalar.activation(u_sb[:], Ru, AF.Tanh, bias=bef1c_sb[:, 0:1])
                yield
                # mj = 1 - sigmoid(u @ Wef3 + bef3) = sigmoid(-z - bef3)
                nc.tensor.matmul(Rj0, Wef3_sb[:, 0:P], u_sb[:],
                                 start=True, stop=True, skip_group_check=True)
                yield
                nc.scalar.activation(jmj[par][:, :, 0, :],
                                     Rj0.rearrange("p (t b) -> p t b", b=B_LOC),
                                     AF.Sigmoid, scale=-1.0, bias=nbef3_sb[:, 0:1])
                yield
                nc.tensor.matmul(Rj1, Wef3_sb[:, P:2 * P], u_sb[:],
                                 start=True, stop=True, skip_group_check=True)
                yield
                nc.scalar.activation(jmj[par][:, :, 1, :],
                                     Rj1.rearrange("p (t b) -> p t b", b=B_LOC),
                                     AF.Sigmoid, scale=-1.0, bias=nbef3_sb[:, 1:2])
                yield
                # jh = j/2 = -0.5*mj + 0.5
                nc.vector.tensor_scalar(jmj[par][:, :, 2, :], jmj[par][:, :, 0, :],
                                        -0.5, 0.5, op0=OP.mult, op1=OP.add)
                yield
                nc.vector.tensor_scalar(jmj[par][:, :, 3, :], jmj[par][:, :, 1, :],
                                        -0.5, 0.5, op0=OP.mult, op1=OP.add)
                yield

                # phase B: per bank, ONE bank-wide start=True zeroing matmul,
                # then bias + x@Wx accumulate with start=False.
                xf = xT[par][:].rearrange("p t b -> p (t b)")
                for bk in range(4):
                    nc.tensor.matmul(
                        g[:, 2 * bk:2 * bk + 2, :, :], zrow[:], ones_bf[:],
                        start=True, stop=False, skip_group_check=True)
                    yield
                for blk in range(8):
                    co = BLK_COL[blk]
                    nc.tensor.matmul(g[:, blk, :, :], biass_sb[:, co:co + P],
                                     ones_bf[:, 0:NC], start=False, stop=False,
                                     skip_group_check=True)
                    nc.tensor.matmul(g[:, blk, :, :], Wxs_sb[:, co:co + P], xf,
                                     start=False, stop=False, skip_group_check=True)
                    yield

            def scan(par, gen=None, nsteps=mc):
                """Sequential scan over one chunk's mc steps. `gen` is an
                optional phases() generator whose emission is interleaved
                one slot per step (its PSUM/engine deps are already free)."""
                g = gts[par]
                jm = jmj[par]
                for t in range(nsteps):
                    # MV holds [m1 | v] per half; the gate matmuls stream both
                    # and PSUM-accumulate, so h = m1 + v is never formed on
                    # the critical chain.
                    MV = scr.tile([P, 2, 2, B_LOC], bf16, tag="MV")
                    # v = (1-j) * h_{t-1}  (off-chain, reads hm = u+v of t-1)
                    nc.gpsimd.tensor_mul(MV[:, :, 1, :], jm[:, t, 0:2, :], hT[:])
                    # peephole diag matmuls for i,f; then the critical i,f,g
                    # h-matmuls; o-side last so tanh_ifg doesn't wait on it.
                    for d, blk in zip((0, 1, 2, 3), (0, 1, 2, 3)):
                        nc.tensor.matmul(g[:, blk, t, :], dWc_sb[:, d, :],
                                         TS[:, 6 + (d % 2):7 + (d % 2), :],
                                         start=False, stop=False,
                                         skip_group_check=True)
                    MVp = MV_prev[0]
                    for blk in (0, 1, 2, 3, 4, 5):
                        co = BLK_COL[blk]
                        for k in range(2):
                            nc.tensor.matmul(g[:, blk, t, :].unsqueeze(1)
                                             .to_broadcast([P, 2, B_LOC]),
                                             Whs_sb[:, k, co:co + P],
                                             MVp[:, k, :, :],
                                             start=False, stop=False,
                                             skip_group_check=True)
                    for d, blk in zip((4, 5), (6, 7)):
                        nc.tensor.matmul(g[:, blk, t, :], dWc_sb[:, d, :],
                                         TS[:, 6 + (d % 2):7 + (d % 2), :],
                                         start=False, stop=False,
                                         skip_group_check=True)
                    for blk in (6, 7):
                        co = BLK_COL[blk]
                        for k in range(2):
                            nc.tensor.matmul(g[:, blk, t, :].unsqueeze(1)
                                             .to_broadcast([P, 2, B_LOC]),
                                             Whs_sb[:, k, co:co + P],
                                             MVp[:, k, :, :],
                                             start=False, stop=(k == 1),
                                             skip_group_check=True)
                    # all gates via one function: t = tanh(bank)
                    nc.scalar.activation(TS[:, 0:6, :], g[:, 0:6, t, :], AF.Tanh)
                    nc.scalar.activation(TS[:, 10:12, :], g[:, 6:8, t, :], AF.Tanh)
                    # A = (t_if + 1) * [g, c] ; ch2 = 2*c_hat = A[0:2] + A[2:4]
                    A_t = scr.tile([P, 4, B_LOC], bf16, tag="A")
                    nc.vector.scalar_tensor_tensor(A_t[:], TS[:, 0:4, :], 1.0,
                                                   TS[:, 4:8, :],
                                                   op0=OP.add, op1=OP.mult)
                    nc.vector.tensor_add(TS[:, 8:10, :], A_t[:, 0:2, :], A_t[:, 2:4, :])
                    # th = tanh(c_hat) = tanh(0.5 * ch2)
                    th_t = scr.tile([P, 2, B_LOC], bf16, tag="th")
                    nc.scalar.activation(th_t[:], TS[:, 8:10, :], AF.Tanh, scale=0.5)
                    # jo = (t_o + 1) * jh = j*o  (off-chain)
                    jo_t = scr.tile([P, 2, B_LOC], bf16, tag="jo")
                    nc.vector.scalar_tensor_tensor(jo_t[:], TS[:, 10:12, :], 1.0,
                                                   jm[:, t, 2:4, :],
                                                   op0=OP.add, op1=OP.mult)
                    # c' = mj*c + jh*ch2  (off-chain)
                    ccm_t = scr.tile([P, 4, B_LOC], bf16, tag="ccm")
                    nc.gpsimd.tensor_mul(ccm_t[:], jm[:, t, :, :], TS[:, 6:10, :])
                    nc.gpsimd.tensor_add(TS[:, 6:8, :], ccm_t[:, 0:2, :], ccm_t[:, 2:4, :])
                    # m1 = jo*th  (chain end; next step's matmuls stream MV)
                    nc.vector.tensor_mul(MV[:, :, 0, :], jo_t[:], th_t[:])
                    # hm = m1 + v  (off-chain; feeds next v and final output)
                    nc.vector.tensor_add(hT[:], MV[:, :, 0, :], MV[:, :, 1, :])
                    MV_prev[0] = MV
                    if gen is not None:
                        next(gen, None)
                if gen is not None:
                    for _ in gen:
                        pass

            # ---------------- prologue ----------------
            for _ in phases(0, 0):
                pass

            if dbg_steps is not None:
                for _ in phases(mc, 1):
                    pass
                scan(0, None, dbg_steps)
                stg = stp.tile([P, 12, B_LOC], fp32, name="stg")
                nc.vector.tensor_scalar_add(stg[:], TS[:], 0.0)
                nc.sync.dma_start(ts_dbg_d[:], stg[:])
                stg2 = stp.tile([P, 2, B_LOC], fp32, name="stg2")
                nc.vector.tensor_scalar_add(stg2[:], hT[:], 0.0)
                nc.sync.dma_start(h_dbg_d[:], stg2[:])
                stg3 = stp.tile([P, mc, B_LOC], fp32, name="stg3")
                nc.vector.tensor_scalar_add(stg3[:], xT[0][:], 0.0)
                nc.sync.dma_start(x_dbg_d[:], stg3[:])
                stg4 = stp.tile([P, mc, 4, B_LOC], fp32, name="stg4")
                nc.vector.tensor_scalar_add(stg4[:], jmj[0][:], 0.0)
                nc.sync.dma_start(j_dbg_d[:], stg4[:])
                stg5 = stp.tile([P, 8, B_LOC], fp32, name="stg5")
                nc.vector.tensor_scalar_add(stg5[:], gts[0][:, :, max(dbg_steps - 1, 0), :], 0.0)
                nc.sync.dma_start(g_dbg_d[:], stg5[:])

            if dbg_steps is None:
                # ---------------- main loop ----------------
                def body(i):
                    # scan chunk 2i while emitting phases(2i+1) into its
                    # per-step stream; then chunk 2i+1 with phases(2i+2).
                    scan(0, phases(i * (2 * mc) + mc, 1))
                    scan(1, phases(i * (2 * mc) + 2 * mc, 0))

                if n_body > 0:
                    with tc.For_i(0, n_body, 1,
                                  hint_engines=(mybir.EngineType.PE,
                                                mybir.EngineType.Activation,
                                                mybir.EngineType.DVE,
                                                mybir.EngineType.Pool)) as i:
                        body(i)

                # ---------------- epilogue: last two chunks ----------------
                scan(0, phases((n_chunks - 1) * mc, 1))
                scan(1, None)

                # ---------------- output projection ----------------
                ps_o = psp.tile([DIM, B_LOC], fp32, tag="g0")
                for k in range(2):
                    nc.tensor.matmul(ps_o[:], Wlins_sb[:, k, :], hT[:, k, :],
                                     start=(k == 0), stop=(k == 1),
                                     skip_group_check=True)
                outT = stp.tile([DIM, B_LOC], fp32)
                nc.scalar.activation(outT[:], ps_o[:], AF.Identity, bias=blinc_sb[:, 0:1])
                nc.sync.dma_start(out_d.rearrange("b d -> d b"), outT[:])

    nc.finalize()
    return nc


_NC_CACHE = {}


def _get_nc(s_total=S, mc=MC, dbg_steps=None):
    key = (s_total, mc, dbg_steps)
    if key not in _NC_CACHE:
        _NC_CACHE[key] = build_nc(s_total, mc, dbg_steps)
    return _NC_CACHE[key]


def run_dbg(inputs, s_total, dbg_steps):
    from concourse.bass_utils import run_bass_kernel_spmd

    nc = _get_nc(s_total, MC, dbg_steps)
    in_maps = _make_in_maps(inputs, s_total)
    res = run_bass_kernel_spmd(nc, in_maps[:1], [0])
    return res.results[0]


def _prep_shared(inputs):
    import concourse.mybir as mybir
    bf = mybir.dt.np(mybir.dt.bfloat16)

    colscale = np.full((G4,), 0.5, np.float32)
    colscale[2 * HS:3 * HS] = 1.0          # g columns unscaled

    Wh = np.asarray(inputs["Wh"], np.float32) * colscale
    Whs = np.ascontiguousarray(Wh.reshape(2, P, G4).transpose(1, 0, 2)).astype(bf)
    Wxs = np.ascontiguousarray(np.asarray(inputs["Wx"], np.float32) * colscale)
    biass = (np.asarray(inputs["bias"], np.float32) * colscale).reshape(1, G4).astype(bf)

    Wc = np.asarray(inputs["Wc"], np.float32)
    dWc = np.zeros((P, 6, P), np.float32)
    for d, (gi, hf) in enumerate(DIAG_SRC):
        seg = Wc[gi, hf * P:(hf + 1) * P] * 0.5
        dWc[np.arange(P), d, np.arange(P)] = seg
    dWc = dWc.astype(bf)

    Wlin = np.asarray(inputs["Wlin"], np.float32)
    Wlins = np.ascontiguousarray(Wlin.reshape(2, P, DIM).transpose(1, 0, 2)).astype(bf)

    return {
        "Whs": Whs, "Wxs": Wxs, "biass": biass, "dWc": dWc, "Wlins": Wlins,
        "Ve": np.ascontiguousarray(inputs["Ve"], np.float32),
        "Vc2": np.ascontiguousarray(2.0 * np.asarray(inputs["Vc"], np.float32)),
        "Vn": np.ascontiguousarray(inputs["Vn"], np.float32),
        "Wef1": np.ascontiguousarray(inputs["Wef1"], np.float32),
        "Wef3": np.ascontiguousarray(inputs["Wef3"], np.float32),
        "bef1c": np.ascontiguousarray(np.asarray(inputs["bef1"], np.float32).reshape(EF, 1)),
        "nbef3": np.ascontiguousarray((-np.asarray(inputs["bef3"], np.float32)).reshape(2, P).T),
        "blinc": np.ascontiguousarray(np.asarray(inputs["blin"], np.float32).reshape(DIM, 1)),
    }


def _make_in_maps(inputs, s_total=S):
    import concourse.mybir as mybir
    bf = mybir.dt.np(mybir.dt.bfloat16)
    shared = _prep_shared(inputs)

    def stateT(x):   # [B_LOC, 256] -> [128, 2, B_LOC] bf16
        return np.ascontiguousarray(
            x.T.reshape(2, P, B_LOC).transpose(1, 0, 2)).astype(bf)

    per_core = []
    for i in range(N_CORES):
        sl = slice(i * B_LOC, (i + 1) * B_LOC)
        m = dict(shared)
        m["event"] = np.ascontiguousarray(inputs["event"][sl, :s_total], np.float32)
        m["vc"] = np.ascontiguousarray(inputs["vc"][sl, :s_total], np.float32)
        m["vn"] = np.ascontiguousarray(inputs["vn"][sl, :s_total], np.float32)
        m["h0T"] = stateT(np.asarray(inputs["h0"], np.float32)[sl])
        m["c0T"] = stateT(np.asarray(inputs["c0"], np.float32)[sl])
        per_core.append(m)
    return per_core


def run(inputs, s_total=S, mc=MC, trace=False):
    """Returns (out [B_FULL, DIM], exec_time_ns or None)."""
    from concourse.bass_utils import run_bass_kernel_spmd

    nc = _get_nc(s_total, mc)
    in_maps = _make_in_maps(inputs, s_total)
    res = run_bass_kernel_spmd(nc, in_maps, list(range(N_CORES)), trace=trace)
    out = np.concatenate([res.results[i]["out"] for i in range(N_CORES)], axis=0)
    return out, res.exec_time_ns


def kernel(**inputs):
    out, _ = run(inputs)
    return out
